# revision 17
# baseline (speedup 1.0000x reference)
"""Trainium2 Bass kernel for nn_CRFillModule (patch cosine-similarity attention).

reference math (B=2, C=192, H=W=48, ps=3, stride=1 -> N=46*46=2116 patches, K=9):
  p = l2_normalize(extract_patches(feat), axis=K)        # [B,C,N,K]
  q = l2_normalize(extract_patches(feat*(1-mask)), axis=K)
  sim  = einsum('bcnk,bcmk->bnm', q, p) * 10.0           # contraction D = C*K = 1728
  attn = softmax(sim, -1)
  recon_p = einsum('bnm,bcmk->bcnk', attn, p) -> [B,C,46,46,3,3]
  recon = zeros_like(feat); cr_loss = mean((feat*(1-mask))**2)

Sharding: data-parallel over B (2) x query-row quarters (4) = 8 cores.
Each core runs the full attention for its 529 query rows, flash-style over
query blocks of 128: QK matmuls (fp16 in, fp32 psum) -> row softmax (DVE
chunk-max overlapped with QK, ACT exp + fused row-sum) -> PE transpose of
attn -> AV matmuls -> 1/rowsum scaling -> DMA out. The AV matmuls of block
i-1 are emitted between QK(i) and transpose(i) so they fill the PE gap
while softmax(i) runs on DVE/ACT. Host does layout only (patch gather,
l2 norm, fp16 cast, padding).
"""

import numpy as np

B, C, H, W = 2, 192, 48, 48
PS, SCALE, EPS = 3, 10.0, 1e-12
HO = WO = H - PS + 1          # 46
N = HO * WO                   # 2116 patches per image
K = PS * PS                   # 9
D = C * K                     # 1728 contraction
NT_D = 14                     # ceil(1728/128) -> D padded to 1792
DP = NT_D * 128
NT_M = 17                     # ceil(2116/128) key tiles
NQ = N // 4                   # 529 query rows per core
N_CORES = 8
QBLKS = [(i * 128, min(128, NQ - i * 128)) for i in range((NQ + 127) // 128)]
MT_LEN = [min(128, N - mt * 128) for mt in range(NT_M)]
MCHUNKS = [(i * 512, min(512, N - i * 512)) for i in range((N + 511) // 512)]
AV_HALF = 864                 # 1728 split in 2 psum-resident halves (2 banks each)
TGRP = 8                      # attn transposes batched per 1-bank fp16 psum tile

# fp8e4m3 QK with DoubleRow (2 fp8 MACs/cell/cycle). The softmax is argmax-
# like, so fp8 logits cost no accuracy (verified vs fp32 reference). The
# DoubleRow pair dim steps by the free row length, which must be 16B-aligned
# in fp8 -> pad 529->544 and 2116->2128.
QK_FP8 = True
NQP = 544
NP8 = 2128
MCHUNKS8 = [(i * 512, min(512, NP8 - i * 512)) for i in range((NP8 + 511) // 512)]

_nc_cache = {}


def _build_nc(reps=None, qk_fp8=QK_FP8, load_in_loop=True, qk_only=False,
              dummy_io=False, no_sm=False, no_max=False):
    import contextlib

    import concourse.bacc as bacc
    import concourse.tile as tile
    from concourse import mybir
    from concourse.masks import make_identity

    nc = bacc.Bacc("TRN2", target_bir_lowering=False, debug=False)
    f16, f32 = mybir.dt.float16, mybir.dt.float32
    qdt = mybir.dt.float8e4 if qk_fp8 else f16
    nqp = NQP if qk_fp8 else NQ
    np_ = NP8 if qk_fp8 else N
    mchunks = MCHUNKS8 if qk_fp8 else MCHUNKS

    # dummy_io: timing-only build -- no external IO so the axon RPC carries
    # no tensor payload; compute runs on uninitialized DRAM.
    in_kind = "Internal" if dummy_io else "ExternalInput"
    out_kind = "Internal" if dummy_io else "ExternalOutput"
    qt_d = nc.dram_tensor("qt", [NT_D, 128, nqp], qdt, kind=in_kind)
    pkm_d = nc.dram_tensor("pkm", [NT_D, 128, np_], qdt, kind=in_kind)
    pmk_d = nc.dram_tensor("pmk", [NT_M, 128, D], f16, kind=in_kind)
    out_d = nc.dram_tensor("out", [NQ, D], f32, kind=out_kind)
    marker_d = (nc.dram_tensor("marker", [128, 4], f32, kind="ExternalOutput")
                if dummy_io else None)

    with tile.TileContext(nc) as tc:
        with (
            tc.tile_pool(name="consts", bufs=1) as consts,
            tc.tile_pool(name="attnp", bufs=3) as attnp,
            tc.tile_pool(name="attntp", bufs=3) as attntp,
            tc.tile_pool(name="outp", bufs=3) as outp,
            tc.tile_pool(name="statp", bufs=4) as statp,
            tc.tile_pool(name="simp", bufs=1, space="PSUM") as simp,
            tc.tile_pool(name="trp", bufs=1, space="PSUM") as trp,
            tc.tile_pool(name="avp", bufs=1, space="PSUM") as avp,
        ):
            ident = consts.tile([128, 128], f16)
            make_identity(nc, ident)
            qt_sb = consts.tile([128, NT_D, nqp], qdt)
            pkm_sb = consts.tile([128, NT_D, np_], qdt)
            pmk_sb = consts.tile([128, NT_M, D], f16)

            def emit_loads():
                # qt first (block 0 needs it), then pkm in m-chunk slices so
                # the first QK chunk starts early, then pmk (needed at AV(0)).
                for t in range(NT_D):
                    nc.sync.dma_start(out=qt_sb[:, t, :], in_=qt_d[t])
                for m0, mlen in ((0, 512), (512, 1058 - 512), (1058, np_ - 1058)):
                    for t in range(NT_D):
                        nc.sync.dma_start(
                            out=pkm_sb[:, t, m0:m0 + mlen],
                            in_=pkm_d[t, :, m0:m0 + mlen],
                        )
                for t in range(NT_M):
                    for c0 in (0, AV_HALF):
                        nc.sync.dma_start(
                            out=pmk_sb[:, t, c0:c0 + AV_HALF],
                            in_=pmk_d[t, :, c0:c0 + AV_HALF],
                        )

            pending = []

            def emit_av():
                """AV matmuls + output scale/DMA for the oldest pending block."""
                q0, qlen, attnT, recip = pending.pop(0)
                osb = outp.tile([128, D], f32)
                for c0 in range(0, D, AV_HALF):
                    avt = avp.tile([128, AV_HALF], f32)
                    for mt in range(NT_M):
                        mlen = MT_LEN[mt]
                        lhs = attnT[0:mlen, mt, 0:qlen]
                        for cc0, cclen in ((0, 512), (512, AV_HALF - 512)):
                            nc.tensor.matmul(
                                avt[0:qlen, cc0:cc0 + cclen],
                                lhs,
                                pmk_sb[0:mlen, mt, c0 + cc0:c0 + cc0 + cclen],
                                start=(mt == 0),
                                stop=(mt == NT_M - 1),
                            )
                    nc.scalar.mul(
                        osb[0:qlen, c0:c0 + AV_HALF], avt[0:qlen], recip[0:qlen]
                    )
                nc.sync.dma_start(out=out_d[q0:q0 + qlen, :], in_=osb[0:qlen, :])

            def emit_compute():
                for q0, qlen in QBLKS:
                    # ---- QK: sim[q, m] = sum_d qt[d, q] * pkm[d, m], chunk-
                    # outer with one psum tile per chunk so each chunk's
                    # row-max (DVE) overlaps the next chunk's matmuls (PE).
                    simt = simp.tile([128, np_], f32)
                    for mc, (m0, mlen) in enumerate(mchunks):
                        simc = simt[:, m0:m0 + mlen]
                        if qk_fp8:
                            for tp in range(NT_D // 2):
                                t = 2 * tp
                                nc.tensor.matmul(
                                    simc[0:qlen, 0:mlen],
                                    qt_sb[:, t:t + 2, q0:q0 + qlen],
                                    pkm_sb[:, t:t + 2, m0:m0 + mlen],
                                    start=(tp == 0),
                                    stop=(tp == NT_D // 2 - 1),
                                    perf_mode=mybir.MatmulPerfMode.DoubleRow,
                                )
                        else:
                            for t in range(NT_D):
                                nc.tensor.matmul(
                                    simc[0:qlen, 0:mlen],
                                    qt_sb[:, t, q0:q0 + qlen],
                                    pkm_sb[:, t, m0:m0 + mlen],
                                    start=(t == 0),
                                    stop=(t == NT_D - 1),
                                )
                    if no_sm:
                        continue
                    # ---- softmax over m: single fused -max, single exp with
                    # fused row-sum -- shortest possible dependency chain.
                    if not no_max:
                        negb = statp.tile([128, 1], f32)
                        nc.vector.tensor_reduce(
                            out=negb[0:qlen], in_=simt[0:qlen, 0:N],
                            op=mybir.AluOpType.max, axis=mybir.AxisListType.X,
                            negate=True,
                        )
                        bias_ap = negb[0:qlen]
                    else:
                        bias_ap = 0.0
                    attn = attnp.tile([128, N], f16)
                    rowsum = statp.tile([128, 1], f32)
                    nc.scalar.activation(
                        out=attn[0:qlen], in_=simt[0:qlen, 0:N],
                        func=mybir.ActivationFunctionType.Exp,
                        bias=bias_ap, scale=1.0,
                        accum_out=rowsum[0:qlen],
                    )
                    recip = statp.tile([128, 1], f32)
                    nc.vector.reciprocal(recip[0:qlen], rowsum[0:qlen])
                    if qk_only:
                        continue

                    # AV of the previous block fills the PE pipe while the
                    # softmax of this block runs on DVE/ACT.
                    if pending:
                        emit_av()

                    # ---- transpose attn -> attnT tiles [m_local, q] ----
                    attnT = attntp.tile([128, NT_M, 128], f16)
                    for g0 in range(0, NT_M, TGRP):
                        gts = list(range(g0, min(g0 + TGRP, NT_M)))
                        trt = trp.tile([128, TGRP, 128], f16)
                        gmin = 128
                        for j, mt in enumerate(gts):
                            mlen = MT_LEN[mt]
                            gmin = min(gmin, mlen)
                            nc.tensor.transpose(
                                trt[0:mlen, j, 0:qlen],
                                attn[0:qlen, mt * 128:mt * 128 + mlen],
                                ident[0:qlen, 0:qlen],
                            )
                        nc.vector.tensor_copy(
                            attnT[0:gmin, g0:g0 + len(gts), 0:qlen],
                            trt[0:gmin, 0:len(gts), 0:qlen],
                        )
                    pending.append((q0, qlen, attnT, recip))
                if not qk_only:
                    emit_av()

            if marker_d is not None:
                mk = consts.tile([128, 4], f32)
                nc.vector.memset(mk, 1.0)
                nc.sync.dma_start(out=marker_d[:, :], in_=mk)
                # zero-fill the internal input DRAM so the timed compute sees
                # finite data (exp of garbage -> inf/NaN notifications)
                zq = consts.tile([128, max(nqp, np_)], qdt)
                nc.vector.memset(zq, 0)
                zf = consts.tile([128, D], f16)
                nc.vector.memset(zf, 0)
                for t in range(NT_D):
                    nc.sync.dma_start(out=qt_d[t], in_=zq[:, 0:nqp])
                    nc.sync.dma_start(out=pkm_d[t], in_=zq[:, 0:np_])
                for t in range(NT_M):
                    nc.sync.dma_start(out=pmk_d[t], in_=zf)
            if reps:
                if not load_in_loop:
                    emit_loads()
                with tc.For_i(0, reps, 1):
                    if load_in_loop:
                        emit_loads()
                    emit_compute()
            else:
                emit_loads()
                emit_compute()
    nc.finalize()
    return nc


def _get_nc():
    if "nc" not in _nc_cache:
        _nc_cache["nc"] = _build_nc()
    return _nc_cache["nc"]


def _host_prep(feat, mask, qk_fp8=QK_FP8):
    """-> per-core input maps. Layout/cast only (plus l2 norm)."""
    from numpy.lib.stride_tricks import sliding_window_view

    if qk_fp8:
        import ml_dtypes
        qdt, nqp, np_ = ml_dtypes.float8_e4m3, NQP, NP8
    else:
        qdt, nqp, np_ = np.float16, NQ, N

    in_maps = []
    for b in range(B):
        fp = feat[b]                                   # [C,H,W]
        fq = fp * (1.0 - mask[b])
        mats = []
        for f in (fp, fq):
            p = sliding_window_view(f, (PS, PS), axis=(1, 2))  # [C,HO,WO,3,3]
            p = np.ascontiguousarray(p).reshape(C, N, K)
            nrm = np.sqrt((p.astype(np.float32) ** 2).sum(-1, keepdims=True))
            p = p / np.maximum(nrm, EPS)
            mats.append(p.transpose(1, 0, 2).reshape(N, D))  # [m, (c,k)]
        pD, qD = mats
        # fold the x10 similarity scale into the key matrix so exp() needs
        # no extra scaling (values stay well inside fp16/fp8 range)
        pT = np.zeros((DP, np_), qdt)
        pT[:D, :N] = (pD.T * SCALE).astype(qdt)
        pkm = pT.reshape(NT_D, 128, np_)
        pmk = np.zeros((NT_M * 128, D), np.float16)
        pmk[:N] = pD.astype(np.float16)
        pmk = pmk.reshape(NT_M, 128, D)
        for j in range(4):
            qT = np.zeros((DP, nqp), qdt)
            qT[:D, :NQ] = qD[j * NQ:(j + 1) * NQ].T.astype(qdt)
            in_maps.append({
                "qt": qT.reshape(NT_D, 128, nqp),
                "pkm": pkm,
                "pmk": pmk,
            })
    return in_maps


def _run_device(in_maps, trace=False):
    from concourse.bass_utils import run_bass_kernel_spmd
    return run_bass_kernel_spmd(
        _get_nc(), in_maps, core_ids=list(range(N_CORES)), trace=trace
    )


def kernel(feat, mask):
    feat = np.ascontiguousarray(np.asarray(feat, dtype=np.float32))
    mask = np.ascontiguousarray(np.asarray(mask, dtype=np.float32))

    res = _run_device(_host_prep(feat, mask))

    recon_p = np.empty((B, C, HO, WO, PS, PS), np.float32)
    for core, r in enumerate(res.results):
        b, j = divmod(core, 4)
        blk = r["out"].reshape(NQ, C, K).transpose(1, 0, 2)   # [C, nq, K]
        recon_p.reshape(B, C, N, K)[b, :, j * NQ:(j + 1) * NQ] = blk

    recon = np.zeros_like(feat)
    diff = (recon - feat) * (1.0 - mask)
    cr_loss = np.float32((diff.astype(np.float64) ** 2).mean())
    return recon, cr_loss, recon_p
